# revision 1
# baseline (speedup 1.0000x reference)
"""LSTM kernel for Trainium2 (Bass/Tile), 8-core data-parallel.

Model (per reference):
    xg = einsum('bsd,dg->sbg', x, Wi)            # input projections
    per step: z = xg_t + h @ Wh + bh
              i,f,g,o = split(z); c = sig(f)*c + sig(i)*tanh(g); h = sig(o)*tanh(c)
    out = h_last @ Wo + bo

Sharding: batch 256 -> 32 per core, weights replicated.

On-chip layout (per core):
  - gates-on-partitions: z for one step is a PSUM region [128, 128] laid out as
    [i|f|o|g] x 32 batch columns. Partition p = hidden feature; so i,f,o,g,c,h
    are all [H=128, B=32] tiles and h is directly the next matmul's rhs.
  - xg is precomputed by PE matmuls (lhsT = [Wi; bh] with a ones-row appended to
    x) straight into PSUM chunks of 16 steps; the per-step recurrence matmuls
    accumulate on top with start=False.
"""

import copy

import numpy as np

import concourse.bass as bass
import concourse.mybir as mybir
from concourse import tile
from concourse.bass_utils import run_bass_kernel_spmd

F32 = mybir.dt.float32

B, S, D, H = 256, 4096, 64, 128
G4 = 4 * H  # 512
NCORES = 8
BC = B // NCORES  # 32 batch per core
TC = 16  # timesteps per PSUM chunk (4 banks)
BODY_CH = 4  # chunks per loop body (static x-slot / psum ping-pong)
KD = D + 1  # contraction rows for input projection (ones row folds bh in)
CPC = TC * BC  # x columns per chunk (512)

# on-chip gate block order [i, f, o, g]; reference order is [i, f, g, o]
_PERM = np.concatenate(
    [np.arange(0, 128), np.arange(128, 256), np.arange(384, 512), np.arange(256, 384)]
)


def _legalize_for_walrus(nc):
    """Make the Tile-scheduled module lowerable by this walrus build.

    (1) This walrus accepts only ONE semaphore wait per TPB instruction
        (e.g. Matmult/LDWEIGHTS and DMACopy structs have a single wait slot);
        Tile emits multi-wait instructions. Hoist excess waits onto standalone
        EventSemaphore sequencer instructions placed just before, on the same
        engine — semantically identical (the sequencer blocks in order).
    (2) Drop the trailing EVENT_SEMAPHORE_RANGE_CLEAR InstISA (sem-recycling
        hygiene) which this walrus cannot lower at all.
    """
    f = nc.m.functions[0]
    template = None
    for blk in f.blocks:
        for inst in blk.instructions:
            if type(inst).__name__ == "InstEventSemaphore":
                template = inst
                break
        if template is not None:
            break
    assert template is not None, "no EventSemaphore to clone"
    uid = 0
    for blk in f.blocks:
        out = []
        for inst in blk.instructions:
            nm = type(inst).__name__
            if nm == "InstISA":
                continue  # (2)
            si = inst.sync_info
            waits = list(si.on_wait) if si is not None else []
            if nm != "InstEventSemaphore" and len(waits) > 1:
                for w in waits[1:]:
                    es = copy.deepcopy(template)
                    es.name = f"{inst.name}_hoist{uid}"
                    uid += 1
                    es.engine = inst.engine
                    es.sync_info = mybir.SyncInfo(on_wait=[w], on_update=[])
                    out.append(es)
                inst.sync_info = mybir.SyncInfo(
                    on_wait=waits[:1], on_update=list(si.on_update)
                )
            out.append(inst)
        blk.instructions = out


def build_bass(n_steps=S, legalize=True):
    n_ch = n_steps // TC
    assert n_ch % BODY_CH == 0 and n_steps % TC == 0
    n_iter = n_ch // BODY_CH
    pad_ch = n_ch + BODY_CH
    xcols = pad_ch * CPC

    nc = bass.Bass()
    xt = nc.declare_dram_parameter("xt", [KD, xcols], F32, isOutput=False)
    # combined weights: cols [0:512] = Wh (permuted), cols [512:1024] = [Wi; bh]
    # (rows 65:128 of the right half are zero padding)
    wcb = nc.declare_dram_parameter("wcb", [H, 2 * G4], F32, isOutput=False)
    hout = nc.declare_dram_parameter("h_out", [H, BC], F32, isOutput=True)

    with tile.TileContext(nc) as tc:
        with (
            tc.tile_pool(name="weights", bufs=1) as wpool,
            tc.tile_pool(name="xin", bufs=1) as xpool,
            tc.tile_pool(name="state", bufs=1) as spool,
            tc.tile_pool(name="psum", bufs=1, space=bass.MemorySpace.PSUM) as ppool,
        ):
            w_sb = wpool.tile([H, 2 * G4], F32, tag="w")
            wh_sb = w_sb[:, 0:G4]
            wi_sb = w_sb[:KD, G4 : 2 * G4]
            xs_all = xpool.tile([KD, BODY_CH * CPC], F32, tag="xs")
            xs = [xs_all[:, k * CPC : (k + 1) * CPC] for k in range(BODY_CH)]
            # persistent state: [i|f|o|g|c] so that [i|f] and [g|c] are each
            # contiguous 64-col spans (one fused tensor_tensor covers u=i*g, v=f*c)
            st = spool.tile([H, 160], F32, tag="st")
            wk = spool.tile([H, 96], F32, tag="wk")  # [u|v|tanh_c]
            h_sb = spool.tile([H, BC], F32, tag="h")
            ps = [
                ppool.tile([H, TC * 128], F32, tag=f"ps{k}", name=f"ps{k}")
                for k in range(2)
            ]

            # chunk layout per psum tile: [bank q (4)][gate block gb (4)][t (4)][b (32)]
            # so each xg matmul writes one contiguous [128, 128] in-bank region.
            def xg_chunk(p, xsrc):
                """Input-projection matmuls for one 16-step chunk into psum tile p."""
                for gb in range(4):
                    lhsT = wi_sb[:, gb * H : (gb + 1) * H]
                    for q in range(TC // 4):  # one matmul per PSUM bank
                        nc.tensor.matmul(
                            p[:, q * 512 + gb * 128 : q * 512 + (gb + 1) * 128],
                            lhsT,
                            xsrc[:, q * 4 * BC : (q + 1) * 4 * BC],
                            start=(gb == 0),
                            stop=False,
                            skip_group_check=True,
                        )

            def step(p, j):
                """One LSTM timestep; z for step j=4q+r is strided inside bank q."""
                q, r = j // 4, j % 4
                zoff = q * 512 + r * BC
                for gb in range(4):
                    nc.tensor.matmul(
                        p[:, zoff + gb * 128 : zoff + gb * 128 + BC],
                        wh_sb[:, gb * H : (gb + 1) * H],
                        h_sb[:, :],
                        start=False,
                        stop=True,
                        skip_group_check=True,
                    )
                act = mybir.ActivationFunctionType
                # strided views: gates i,f,o (and g) for step j sit 128 apart
                pz = p[:].rearrange("p (q gb z) -> p q gb z", q=4, gb=4)[:, q, :, :]
                # sigmoid over [i|f|o], tanh over g (PSUM -> SBUF)
                nc.scalar.activation(
                    st[:].rearrange("p (a z) -> p a z", z=BC)[:, 0:3, :],
                    pz[:, 0:3, r * BC : (r + 1) * BC],
                    act.Sigmoid,
                )
                nc.scalar.activation(
                    st[:, 96:128], pz[:, 3, r * BC : (r + 1) * BC], act.Tanh
                )
                # [u|v] = [i|f] * [g|c]
                nc.vector.tensor_mul(wk[:, 0:64], st[:, 0:64], st[:, 96:160])
                # c = u + v
                nc.vector.tensor_add(st[:, 128:160], wk[:, 0:32], wk[:, 32:64])
                nc.scalar.activation(wk[:, 64:96], st[:, 128:160], act.Tanh)
                # h = o * tanh(c)
                nc.vector.tensor_mul(h_sb[:, :], st[:, 64:96], wk[:, 64:96])

            def rec_chunk(p):
                for j in range(TC):
                    step(p, j)

            # ---- preamble ----
            nc.sync.dma_start(w_sb[:], wcb[:])
            nc.vector.memset(h_sb[:], 0.0)
            nc.vector.memset(st[:, 128:160], 0.0)  # c = 0
            nc.sync.dma_start(xs_all[:], xt[:, 0 : BODY_CH * CPC])
            xg_chunk(ps[0], xs[0])
            xg_chunk(ps[1], xs[1])

            # ---- main loop: body covers chunks 4i .. 4i+3 ----
            with tc.For_i(
                0, n_iter, 1, hint_engines=(mybir.EngineType.PE,)
            ) as iv:
                base = iv * (BODY_CH * CPC)

                rec_chunk(ps[0])        # chunk 4i
                xg_chunk(ps[0], xs[2])  # chunk 4i+2
                rec_chunk(ps[1])        # chunk 4i+1
                xg_chunk(ps[1], xs[3])  # chunk 4i+3
                # one DMA refills all four slots (chunks 4i+4 .. 4i+7); its WAR
                # on the slot-2/3 reads above orders it mid-body automatically
                nc.sync.dma_start(
                    xs_all[:], xt[:, bass.ds(base + BODY_CH * CPC, BODY_CH * CPC)]
                )
                rec_chunk(ps[0])        # chunk 4i+2
                xg_chunk(ps[0], xs[0])  # chunk 4i+4
                rec_chunk(ps[1])        # chunk 4i+3
                xg_chunk(ps[1], xs[1])  # chunk 4i+5

            nc.sync.dma_start(hout[:], h_sb[:])

    if legalize:  # CoreSim can't run the post-hoc clones; HW compile needs them
        _legalize_for_walrus(nc)
    return nc


def host_inputs(x, Wi, Wh, bh, n_steps=S):
    """Per-core input maps: transposed/padded x, permuted weights."""
    n_ch = n_steps // TC
    pad_ch = n_ch + BODY_CH
    xcols = pad_ch * CPC
    wcb = np.zeros((H, 2 * G4), np.float32)
    wcb[:, 0:G4] = Wh[:, _PERM]
    wcb[0:D, G4:] = Wi[:, _PERM]
    wcb[D, G4:] = bh[_PERM]
    nb = x.shape[0] // NCORES
    in_maps = []
    for core in range(NCORES):
        xc = x[core * nb : (core + 1) * nb]  # [BC, n_steps, D]
        xtc = np.ascontiguousarray(xc.transpose(2, 1, 0)).reshape(D, n_steps * nb)
        full = np.zeros((KD, xcols), np.float32)
        full[:D, : n_steps * nb] = xtc
        full[D, :] = 1.0
        in_maps.append({"xt": full, "wcb": wcb})
    return in_maps


_CACHE = {}


def _run(x, Wi, Wh, bh, trace=False):
    x = np.asarray(x, np.float32)
    if "nc" not in _CACHE:
        _CACHE["nc"] = build_bass()
    nc = _CACHE["nc"]
    in_maps = host_inputs(x, Wi, Wh, bh)
    res = run_bass_kernel_spmd(nc, in_maps, list(range(NCORES)), trace=trace)
    h_full = np.concatenate(
        [np.asarray(res.results[c]["h_out"]).T for c in range(NCORES)], axis=0
    )  # [B, H]
    return h_full, res


def kernel(x, Wi, Wh, bh, Wo, bo):
    x = np.asarray(x, np.float32)
    Wi = np.asarray(Wi, np.float32)
    Wh = np.asarray(Wh, np.float32)
    bh = np.asarray(bh, np.float32)
    Wo = np.asarray(Wo, np.float32)
    bo = np.asarray(bo, np.float32)
    h_full, _ = _run(x, Wi, Wh, bh)
    return (h_full @ Wo + bo).astype(np.float32)



# revision 3
# speedup vs baseline: 1.1474x; 1.1474x over previous
"""LSTM kernel for Trainium2 (Bass/Tile), 8-core data-parallel. v2.

Model (per reference):
    xg = einsum('bsd,dg->sbg', x, Wi)            # input projections
    per step: z = xg_t + h @ Wh + bh
              i,f,g,o = split(z); c = sig(f)*c + sig(i)*tanh(g); h = sig(o)*tanh(c)
    out = h_last @ Wo + bo

Sharding: batch 256 -> 32 per core, weights replicated.

v2 changes vs v1 (the per-step serial chain is the whole cost; every step
is latency-bound, engines mostly idle):
  - all matmul operands in bf16 (fp32 PE matmul is 4 cycles/row, bf16 is 1;
    recurrence error from bf16 inputs measured at ~2e-3 rel, gate is 2e-2)
  - g-gate matmul issued FIRST each step so ACT's tanh(g) overlaps the
    i/f/o matmuls; one fused sigmoid over [i|f|o]
  - elementwise intermediates placed to cut ACT/DVE SBUF access penalty:
    ACT tanh outputs + c live in PSUM (ACT PSUM access 172cyc vs SBUF 222);
    each DVE op keeps <=1 PSUM operand (single PSUM port)
  - TC=4 steps/chunk so one PSUM bank holds a chunk of z
  - double-buffered (step-parity) gate/c/h tiles so cross-step WAR
    dependencies never bind

On-chip layout (per core):
  - gates-on-partitions: z for one step is [128 hidden, 4 gates x 32 batch]
    inside a chunk tile [128, TC*128], gate-major: col gb*128 + t*32 + b,
    gate order [i|f|o|g].
  - per-parity PSUM tile tgc [tg|c|tc] so mul/add/tanh chain stays in PSUM.
"""

import copy

import numpy as np
import ml_dtypes

import concourse.bass as bass
import concourse.mybir as mybir
from concourse import tile
from concourse.bass_utils import run_bass_kernel_spmd

F32 = mybir.dt.float32
BF16 = mybir.dt.bfloat16
NP_BF16 = np.dtype(ml_dtypes.bfloat16)

B, S, D, H = 256, 4096, 64, 128
G4 = 4 * H  # 512
NCORES = 8
BC = B // NCORES  # 32 batch per core
TC = 4  # timesteps per PSUM chunk (1 bank)
BODY_CH = 4  # chunks per loop body (static x-slot / psum ping-pong)
KD = D + 1  # contraction rows for input projection (ones row folds bh in)
CPC = TC * BC  # x columns per chunk (128)

# on-chip gate block order [i, f, o, g]; reference order is [i, f, g, o]
_PERM = np.concatenate(
    [np.arange(0, 128), np.arange(128, 256), np.arange(384, 512), np.arange(256, 384)]
)


def _legalize_for_walrus(nc):
    """Make the Tile-scheduled module lowerable by this walrus build.

    (1) This walrus accepts only ONE semaphore wait per TPB instruction;
        Tile emits multi-wait instructions. Hoist excess waits onto standalone
        EventSemaphore sequencer instructions placed just before, on the same
        engine — semantically identical (the sequencer blocks in order).
    (2) Drop the trailing EVENT_SEMAPHORE_RANGE_CLEAR InstISA (sem-recycling
        hygiene) which this walrus cannot lower at all.
    """
    f = nc.m.functions[0]
    template = None
    for blk in f.blocks:
        for inst in blk.instructions:
            if type(inst).__name__ == "InstEventSemaphore":
                template = inst
                break
        if template is not None:
            break
    assert template is not None, "no EventSemaphore to clone"
    uid = 0
    for blk in f.blocks:
        out = []
        for inst in blk.instructions:
            nm = type(inst).__name__
            if nm == "InstISA":
                continue  # (2)
            si = inst.sync_info
            waits = list(si.on_wait) if si is not None else []
            if nm != "InstEventSemaphore" and len(waits) > 1:
                for w in waits[1:]:
                    es = copy.deepcopy(template)
                    es.name = f"{inst.name}_hoist{uid}"
                    uid += 1
                    es.engine = inst.engine
                    es.sync_info = mybir.SyncInfo(on_wait=[w], on_update=[])
                    out.append(es)
                inst.sync_info = mybir.SyncInfo(
                    on_wait=waits[:1], on_update=list(si.on_update)
                )
            out.append(inst)
        blk.instructions = out
    return nc


def build_bass(n_steps=S, legalize=True):
    n_ch = n_steps // TC
    assert n_ch % BODY_CH == 0 and n_steps % TC == 0
    n_iter = n_ch // BODY_CH
    pad_ch = n_ch + BODY_CH
    xcols = pad_ch * CPC

    act = mybir.ActivationFunctionType

    nc = bass.Bass()
    xt = nc.declare_dram_parameter("xt", [KD, xcols], BF16, isOutput=False)
    # combined weights (bf16): cols [0:512] = Wh (permuted),
    # cols [512:1024] = [Wi; bh] (rows 65:128 of the right half zero padding)
    wcb = nc.declare_dram_parameter("wcb", [H, 2 * G4], BF16, isOutput=False)
    hout = nc.declare_dram_parameter("h_out", [H, BC], F32, isOutput=True)

    with tile.TileContext(nc) as tc:
        with (
            tc.tile_pool(name="weights", bufs=1) as wpool,
            tc.tile_pool(name="xin", bufs=1) as xpool,
            tc.tile_pool(name="state", bufs=1) as spool,
            tc.tile_pool(name="psum", bufs=1, space=bass.MemorySpace.PSUM) as ppool,
        ):
            w_sb = wpool.tile([H, 2 * G4], BF16, tag="w")
            wh_sb = w_sb[:, 0:G4]
            wi_sb = w_sb[:KD, G4 : 2 * G4]
            xs_all = xpool.tile([KD, BODY_CH * CPC], BF16, tag="xs")
            xs = [xs_all[:, k * CPC : (k + 1) * CPC] for k in range(BODY_CH)]
            # per-parity fp32 SBUF tiles: gates [i|f|o] and uv [u|v]
            gates = spool.tile([H, 2 * 3 * BC], F32, tag="gates")
            uv = spool.tile([H, 2 * 2 * BC], F32, tag="uv")
            # h double-buffered by step parity, bf16 (matmul rhs)
            h_bf = spool.tile([H, 2 * BC], BF16, tag="h")
            hf = spool.tile([H, BC], F32, tag="hf")
            # per-parity PSUM [tg|c|tc]
            tgc = ppool.tile([H, 2 * 3 * BC], F32, tag="tgc", name="tgc")
            ps = [
                ppool.tile([H, TC * 128], F32, tag=f"ps{k}", name=f"ps{k}")
                for k in range(2)
            ]

            def tg_sl(p_):  # tanh(g) slot, parity p_
                return tgc[:, p_ * 96 : p_ * 96 + 32]

            def c_sl(p_):
                return tgc[:, p_ * 96 + 32 : p_ * 96 + 64]

            def tc_sl(p_):
                return tgc[:, p_ * 96 + 64 : p_ * 96 + 96]

            def g_sl(p_):  # gates [i|f|o] parity slice
                return gates[:, p_ * 96 : (p_ + 1) * 96]

            def uv_sl(p_):
                return uv[:, p_ * 64 : (p_ + 1) * 64]

            def h_sl(p_):
                return h_bf[:, p_ * BC : (p_ + 1) * BC]

            # chunk layout: [gate block gb (4) x t (TC) x b (32)], so each xg
            # matmul writes one contiguous [128, 128] region.
            def xg_chunk(p, xsrc):
                """Input-projection matmuls for one TC-step chunk into psum p.

                start=True only on the first matmul: start marks the whole
                2KB zero region (bank) pending-zero, so a second start=True
                would discard the earlier gate blocks' values."""
                for gb in range(4):
                    nc.tensor.matmul(
                        p[:, gb * 128 : (gb + 1) * 128],
                        wi_sb[:, gb * H : (gb + 1) * H],
                        xsrc[:, :],
                        start=(gb == 0),
                        stop=False,
                        skip_group_check=True,
                    )

            def step(p, j, t):
                """One LSTM timestep; t is the absolute step index (parity)."""
                P = t % 2  # this step's parity
                Q = 1 - P  # next step's parity (c written for t+1)
                joff = j * BC
                hP = h_sl(P)
                # recurrence matmuls, g first so tanh(g) starts early
                for gb in (3, 0, 1, 2):
                    nc.tensor.matmul(
                        p[:, gb * 128 + joff : gb * 128 + joff + BC],
                        wh_sb[:, gb * H : (gb + 1) * H],
                        hP,
                        start=False,
                        stop=True,
                        skip_group_check=True,
                    )
                # tanh(g): PSUM -> PSUM
                nc.scalar.activation(
                    tg_sl(P), p[:, 3 * 128 + joff : 3 * 128 + joff + BC], act.Tanh
                )
                # sigmoid over [i|f|o]: PSUM (strided) -> SBUF
                pz = p[:].rearrange("p (gb x) -> p gb x", gb=4)
                gv = g_sl(P).rearrange("p (a x) -> p a x", x=BC)
                nc.scalar.activation(
                    gv[:, :, :], pz[:, 0:3, joff : joff + BC], act.Sigmoid
                )
                # [u|v] = [i|f] * [tg|c]   (one PSUM source)
                nc.vector.tensor_mul(
                    uv_sl(P), g_sl(P)[:, 0:64], tgc[:, P * 96 : P * 96 + 64]
                )
                # c' = u + v  (written to the other parity's c slot)
                nc.vector.tensor_add(c_sl(Q), uv_sl(P)[:, 0:32], uv_sl(P)[:, 32:64])
                # tanh(c'): PSUM -> PSUM
                nc.scalar.activation(tc_sl(P), c_sl(Q), act.Tanh)
                # h = o * tanh(c)  -> bf16, other parity (read by step t+1)
                nc.vector.tensor_mul(h_sl(Q), g_sl(P)[:, 64:96], tc_sl(P))

            def rec_chunk(p, base_t):
                for j in range(TC):
                    step(p, j, base_t + j)

            # ---- preamble ----
            nc.sync.dma_start(w_sb[:], wcb[:])
            nc.vector.memset(h_bf[:], 0.0)
            nc.vector.memset(tgc[:], 0.0)
            nc.sync.dma_start(xs_all[:], xt[:, 0 : BODY_CH * CPC])
            xg_chunk(ps[0], xs[0])
            xg_chunk(ps[1], xs[1])

            # ---- main loop: body covers chunks 4i .. 4i+3 (16 steps) ----
            # TC*BODY_CH = 16 steps per body, even -> parity pattern repeats.
            with tc.For_i(0, n_iter, 1, hint_engines=(mybir.EngineType.PE,)) as iv:
                base = iv * (BODY_CH * CPC)

                rec_chunk(ps[0], 0)          # chunk 4i   (steps 0..3 mod 16)
                xg_chunk(ps[0], xs[2])       # chunk 4i+2
                rec_chunk(ps[1], TC)         # chunk 4i+1
                xg_chunk(ps[1], xs[3])       # chunk 4i+3
                # one DMA refills all four slots (chunks 4i+4 .. 4i+7); its WAR
                # on the slot-2/3 reads above orders it mid-body automatically
                nc.sync.dma_start(
                    xs_all[:], xt[:, bass.ds(base + BODY_CH * CPC, BODY_CH * CPC)]
                )
                rec_chunk(ps[0], 2 * TC)     # chunk 4i+2
                xg_chunk(ps[0], xs[0])       # chunk 4i+4
                rec_chunk(ps[1], 3 * TC)     # chunk 4i+3
                xg_chunk(ps[1], xs[1])       # chunk 4i+5

            # final h lives in parity slot (n_steps % 2) == 0
            nc.scalar.copy(hf[:], h_sl(n_steps % 2))
            nc.sync.dma_start(hout[:], hf[:])

    if legalize:  # CoreSim can't run the post-hoc clones; HW compile needs them
        _legalize_for_walrus(nc)
    return nc


def host_inputs(x, Wi, Wh, bh, n_steps=S):
    """Per-core input maps: transposed/padded bf16 x, permuted bf16 weights."""
    n_ch = n_steps // TC
    pad_ch = n_ch + BODY_CH
    xcols = pad_ch * CPC
    wcb = np.zeros((H, 2 * G4), NP_BF16)
    wcb[:, 0:G4] = Wh[:, _PERM].astype(NP_BF16)
    wcb[0:D, G4:] = Wi[:, _PERM].astype(NP_BF16)
    wcb[D, G4:] = bh[_PERM].astype(NP_BF16)
    nb = x.shape[0] // NCORES
    in_maps = []
    for core in range(NCORES):
        xc = x[core * nb : (core + 1) * nb]  # [BC, n_steps, D]
        xtc = np.ascontiguousarray(xc.transpose(2, 1, 0)).reshape(D, n_steps * nb)
        full = np.zeros((KD, xcols), NP_BF16)
        full[:D, : n_steps * nb] = xtc.astype(NP_BF16)
        full[D, :] = 1.0
        in_maps.append({"xt": full, "wcb": wcb})
    return in_maps


_CACHE = {}


def _run(x, Wi, Wh, bh, trace=False):
    x = np.asarray(x, np.float32)
    if "nc" not in _CACHE:
        _CACHE["nc"] = build_bass()
    nc = _CACHE["nc"]
    in_maps = host_inputs(x, Wi, Wh, bh)
    res = run_bass_kernel_spmd(nc, in_maps, list(range(NCORES)), trace=trace)
    h_full = np.concatenate(
        [np.asarray(res.results[c]["h_out"]).T for c in range(NCORES)], axis=0
    )  # [B, H]
    return h_full, res


def kernel(x, Wi, Wh, bh, Wo, bo):
    x = np.asarray(x, np.float32)
    Wi = np.asarray(Wi, np.float32)
    Wh = np.asarray(Wh, np.float32)
    bh = np.asarray(bh, np.float32)
    Wo = np.asarray(Wo, np.float32)
    bo = np.asarray(bo, np.float32)
    h_full, _ = _run(x, Wi, Wh, bh)
    return (h_full @ Wo + bo).astype(np.float32)
